# revision 22
# baseline (speedup 1.0000x reference)
"""Trainium2 Bass kernel for CausalGNNLayer (per-node-type Linear, MoE-style routing).

Semantics (matching the reference):
    out[n, :] = x[n, :] @ W[node_types[n]].T + b[node_types[n]]
edge_index is unused by the op.

Strategy:
- Host-side routing-aware sharding: stable-sort nodes by type, split each
  type's node list into two halves -> 8 groups (4 types x 2 cores).
- Each core receives its group's rows of x, pre-transposed to [512, P]
  (so the contraction dim lands on SBUF partitions with contiguous DMA),
  plus that single type's weight (transposed to [in, out]) and bias.
- On-device: dense [P,512] @ [512,512] + bias -> [P,512], tiled as 128-node
  blocks with 4 K-tile PSUM accumulation.  Matmuls run in float32r (the
  tensor engine's full-rate fp32 mode, ~1e-4 relative rounding) which makes
  the kernel HBM-bandwidth-bound rather than PE-bound.
- Host scatters the 8 output shards back into the full [N, 512] output.

This does the minimum flops (each node touched by exactly one weight),
unlike the reference's compute-all-4-then-mask.
"""

import numpy as np
from contextlib import ExitStack

import concourse.bass as bass
import concourse.mybir as mybir
import concourse.tile as tile
from concourse.bass_utils import run_bass_kernel_spmd

N_CORES = 8
IN_CH = 512
OUT_CH = 512
NUM_TYPES = 4
P_BLK = 128          # SBUF partition count / node-block size
KT = IN_CH // P_BLK  # 4 contraction tiles
CHUNK_BLKS = 4       # node blocks per x DMA chunk (512 nodes)
XBUFS = 3            # x-chunk prefetch depth
PSBUFS = 4           # PSUM bank ring depth
OBUFS = 4            # output staging depth
XLAYOUT = "flat"     # "pkn": chunk-major contiguous x layout; "flat": [512, P]

# Set by test harness to capture HW profile; kernel works without it.
TRACE = False
LAST_RESULTS = None

_compile_cache: dict = {}

_legal_nop_counter = [0]


def _legalize_waits(nc: bass.Bass) -> None:
    """This walrus codegen only encodes ONE sync wait per engine instruction.
    Tile's scheduler attaches several.  Split: hoist all-but-one wait of any
    multi-wait instruction into preceding same-engine NoOps (one wait each) —
    semantically identical (the engine stalls on each wait in program order)."""
    for fn in nc.m.functions:
        for blk in fn.blocks:
            insts = blk.instructions
            out = []
            changed = False
            for inst in insts:
                si = inst.sync_info
                waits = list(si.on_wait) if si is not None and si.on_wait else []
                if len(waits) > 1:
                    changed = True
                    for w in waits[:-1]:
                        _legal_nop_counter[0] += 1
                        nop = mybir.InstNoOp(
                            name=f"waitsplit-{_legal_nop_counter[0]}",
                            ins=[],
                            outs=[],
                            engine=inst.engine,
                        )
                        nop.sync_info = mybir.SyncInfo(on_wait=[w], on_update=[])
                        out.append(nop)
                    inst.sync_info = mybir.SyncInfo(
                        on_wait=[waits[-1]], on_update=list(si.on_update or [])
                    )
                out.append(inst)
            if changed:
                blk.instructions = out


def _build_bass(P: int) -> bass.Bass:
    """One-core program: out[P,512] = xT.T @ w + bias (same program on all cores)."""
    nc = bass.Bass("TRN2")
    f32 = mybir.dt.float32
    fmm = mybir.dt.float32r  # full-rate PE fp32 mode; ~1e-4 rel rounding

    nblocks = P // P_BLK
    assert nblocks % CHUNK_BLKS == 0, "P must be a multiple of the chunk size"
    nchunks = nblocks // CHUNK_BLKS
    chunk_n = CHUNK_BLKS * P_BLK

    # x arrives as [chunk, partition, k, n]: each partition's per-chunk data
    # is one contiguous 8KB run -> 4x bigger DMA descriptors than the flat
    # [512, P] layout (whose k-tile rows are 2KB runs scattered by 50KB).
    if XLAYOUT == "pkn":
        xT = nc.dram_tensor(
            "xT", [nchunks, P_BLK, KT, chunk_n], fmm, kind="ExternalInput"
        )
        xT_v = [xT.ap()[c] for c in range(nchunks)]
    else:
        xTf = nc.dram_tensor("xT", [IN_CH, P], fmm, kind="ExternalInput")
        _xf = xTf.ap().rearrange("(k p) n -> p k n", p=P_BLK)
        xT_v = [
            _xf[:, :, c * chunk_n : (c + 1) * chunk_n] for c in range(nchunks)
        ]
    w = nc.dram_tensor("w", [IN_CH, OUT_CH], fmm, kind="ExternalInput")
    bias = nc.dram_tensor("bias", [P_BLK, OUT_CH], f32, kind="ExternalInput")
    out = nc.dram_tensor("out", [P, OUT_CH], f32, kind="ExternalOutput")

    w_v = w.ap().rearrange("(k p) o -> p k o", p=P_BLK)

    with ExitStack() as ctx:
        tc = ctx.enter_context(tile.TileContext(nc))
        wp = ctx.enter_context(tc.tile_pool(name="w", bufs=1))
        bp = ctx.enter_context(tc.tile_pool(name="b", bufs=1))
        xp = ctx.enter_context(tc.tile_pool(name="x", bufs=XBUFS))
        pp = ctx.enter_context(tc.tile_pool(name="ps", bufs=PSBUFS, space="PSUM"))
        op = ctx.enter_context(tc.tile_pool(name="o", bufs=OBUFS))

        w_sb = wp.tile([P_BLK, KT, OUT_CH], fmm)
        nc.sync.dma_start(w_sb[:], w_v[:, :, :])
        b_sb = bp.tile([P_BLK, OUT_CH], f32)
        nc.sync.dma_start(b_sb[:], bias.ap())

        for c in range(nchunks):
            pos = c * CHUNK_BLKS
            x_sb = xp.tile([P_BLK, KT, chunk_n], fmm, tag="x")
            nc.sync.dma_start(x_sb[:], xT_v[c])
            for bi in range(CHUNK_BLKS):
                ps = pp.tile([P_BLK, OUT_CH], f32, tag="ps")
                for k in range(KT):
                    nc.tensor.matmul(
                        ps[:],
                        lhsT=x_sb[:, k, bi * P_BLK : (bi + 1) * P_BLK],
                        rhs=w_sb[:, k, :],
                        start=(k == 0),
                        stop=(k == KT - 1),
                    )
                o_sb = op.tile([P_BLK, OUT_CH], f32, tag="o")
                nc.vector.tensor_add(o_sb[:], ps[:], b_sb[:])
                nc.sync.dma_start(
                    out.ap()[(pos + bi) * P_BLK : (pos + bi + 1) * P_BLK, :], o_sb[:]
                )
    _legalize_waits(nc)
    return nc


def _get_compiled(P: int) -> bass.Bass:
    key = (P, XLAYOUT)
    if key not in _compile_cache:
        _compile_cache[key] = _build_bass(P)
    return _compile_cache[key]


def kernel(x, edge_index, node_types, W, b):
    global LAST_RESULTS
    x = np.asarray(x, dtype=np.float32)
    nt = np.asarray(node_types).astype(np.int64)
    W = np.asarray(W, dtype=np.float32)
    b = np.asarray(b, dtype=np.float32)
    N = x.shape[0]

    # Route nodes: stable sort by type, split each type across 2 cores.
    order = np.argsort(nt, kind="stable")
    counts = np.bincount(nt, minlength=NUM_TYPES)
    groups = []
    start = 0
    for t in range(NUM_TYPES):
        c = int(counts[t])
        idx = order[start : start + c]
        start += c
        h = (c + 1) // 2
        groups.append(idx[:h])
        groups.append(idx[h:])

    chunk_n = CHUNK_BLKS * P_BLK
    P = max(1, max(len(g) for g in groups))
    P = ((P + chunk_n - 1) // chunk_n) * chunk_n

    nc = _get_compiled(P)

    in_maps = []
    for gi, g in enumerate(groups):
        t = gi // 2
        xs = np.zeros((P, IN_CH), np.float32)
        if len(g):
            xs[: len(g)] = x[g]
        if XLAYOUT == "pkn":
            xt = np.ascontiguousarray(
                xs.T.reshape(KT, P_BLK, P // chunk_n, chunk_n).transpose(2, 1, 0, 3)
            )
        else:
            xt = np.ascontiguousarray(xs.T)
        in_maps.append(
            {
                "xT": xt,
                "w": np.ascontiguousarray(W[t].T),
                "bias": np.ascontiguousarray(
                    np.broadcast_to(b[t][None, :], (P_BLK, OUT_CH))
                ),
            }
        )

    res = run_bass_kernel_spmd(nc, in_maps, list(range(N_CORES)), trace=TRACE)
    LAST_RESULTS = res

    out = np.empty((N, OUT_CH), np.float32)
    for gi, g in enumerate(groups):
        if len(g):
            out[g] = res.results[gi]["out"][: len(g)]
    return out
